# revision 27
# baseline (speedup 1.0000x reference)
"""Trainium2 Bass kernel for nn_Block_51178830299350 (dense transformer block).

Strategy (8 NeuronCores, single NEFF):
  - Head-tensor-parallel attention: 2 heads/core over all 4096 tokens.
  - LN1: per-core stats on own 512 tokens + tiny AllGather of rsqrt row.
    Mean-subtraction folded into pre-packed qkv weights (host side).
  - fp32r (TF32-like) matmuls everywhere: full PE rate, ~1e-4 rel err.
  - Flash-style transposed scores [k, q]; softmax denominator via a ones
    column appended to V; causal handled by loop bounds + 0/1 diagonal masks.
  - AllToAll (2MB/rank) to redistribute head-sharded attention output to
    token-sharded; proj + FFN run token-parallel with full weights streamed.
"""
import sys

sys.path.insert(0, "/opt/trn_rl_repo")

import numpy as np

B, S, D, H, HD = 2, 2048, 1024, 16, 64
INNER = 4 * D
NCORES = 8
TOK = B * S              # 4096 flat tokens
TOWN = TOK // NCORES     # 512 tokens per core
EPS = 1e-5
_BUILD_CACHE = {}


def round_fp32r(x: np.ndarray) -> np.ndarray:
    """Round fp32 -> fp32r (11-bit mantissa, RTNE), matching TRN2 PE input fmt."""
    v = np.ascontiguousarray(x, dtype=np.float32).view(np.uint32)
    low = v & np.uint32(0xFFF)
    half = np.uint32(0x800)
    rounded = (v & ~np.uint32(0xFFF)).copy()
    up = (low > half) | ((low == half) & (((v >> np.uint32(12)) & np.uint32(1)) != 0))
    rounded[up] += np.uint32(0x1000)
    return rounded.view(np.float32)


def _np_reference(x, mask, sin, cos, ln1_w, ln1_b, w_qkv, w_proj, ln2_w, ln2_b,
                  w_fc1, w_fc2):
    """Slow numpy fallback (only used if inputs violate kernel assumptions)."""
    from scipy.special import erf

    def ln(t, w, b):
        m = t.mean(-1, keepdims=True)
        v = ((t - m) ** 2).mean(-1, keepdims=True)
        return (t - m) / np.sqrt(v + EPS) * w + b

    def rope(t, sin, cos):
        half = t.shape[-1] // 2
        rot = np.concatenate([-t[..., half:], t[..., :half]], -1)
        return t * cos + rot * sin

    b, s, d = x.shape
    hx = ln(x, ln1_w, ln1_b)
    qkv = (hx @ w_qkv).reshape(b, s, 3, H, HD).transpose(2, 0, 3, 1, 4)
    q, k, v = qkv[0], qkv[1], qkv[2]
    q = rope(q, sin, cos)
    k = rope(k, sin, cos)
    att = np.einsum("bhqd,bhkd->bhqk", q, k) / np.sqrt(HD)
    att = np.where(mask, att, -np.inf)
    att = att - att.max(-1, keepdims=True)
    p = np.exp(att)
    p /= p.sum(-1, keepdims=True)
    o = np.einsum("bhqk,bhkd->bhqd", p, v)
    o = o.transpose(0, 2, 1, 3).reshape(b, s, d) @ w_proj
    x = x + o
    h2 = ln(x, ln2_w, ln2_b)
    h2 = h2 @ w_fc1
    h2 = 0.5 * h2 * (1.0 + erf(h2 / np.sqrt(2.0)))
    h2 = h2 @ w_fc2
    return (x + h2).astype(np.float32)


def _build(debug=False, sim=False):
    key = ("nc", debug, sim)
    if key in _BUILD_CACHE:
        return _BUILD_CACHE[key]
    import concourse.bacc as bacc
    import concourse.bass as bass_mod
    import concourse.tile as tile
    from concourse import mybir

    F32 = mybir.dt.float32
    F32R = mybir.dt.float32r
    AF = mybir.ActivationFunctionType

    nc = bacc.Bacc("TRN2", target_bir_lowering=False, debug=False,
                   enable_asserts=False, num_devices=NCORES)

    # ---------------- DRAM parameters (per core) ----------------
    # packed transposed x: [128, (t*8+dt)*512]; (p, t, dt, i) = xT[dt*128+p, t*512+i]
    xtp = nc.dram_tensor("xtp", [128, 8 * 8 * 512], mybir.dt.bfloat16,
                         kind="ExternalInput")
    # own-slice xT (pre-rounded) for stats + residual: [128, dt*512 + i]
    xres = nc.dram_tensor("xres", [128, 8 * 512], F32R, kind="ExternalInput")
    # qkv weights for this core's 2 heads: [128, dt*384 + {Q(128)|K(128)|V(128)}]
    wqkv = nc.dram_tensor("wqkv", [128, 8 * 384], mybir.dt.bfloat16,
                          kind="ExternalInput")
    # proj weights (full): [128, (jb*8+dt)*128 + j]
    wproj = nc.dram_tensor("wproj", [128, 8 * 8 * 128], mybir.dt.bfloat16, kind="ExternalInput")
    # fc1 (full, ln2_w folded): [128, (j*8+dt)*128 + jj]
    wfc1 = nc.dram_tensor("wfc1", [128, 32 * 8 * 128], mybir.dt.bfloat16, kind="ExternalInput")
    # fc2 (full): [128, (d*32+jt)*128 + dd]
    wfc2 = nc.dram_tensor("wfc2", [128, 8 * 32 * 128], mybir.dt.bfloat16, kind="ExternalInput")
    # rope tables [128, S]: head-dim table stacked twice (both local heads),
    # transposed, sign-folded sin (batch-independent)
    cosr = nc.dram_tensor("cosr", [128, S], F32, kind="ExternalInput")
    sinr = nc.dram_tensor("sinr", [128, S], F32, kind="ExternalInput")
    # 4 canonical 0/1 diagonal mask tiles [128k, 512q]
    maskt = nc.dram_tensor("maskt", [128, 4 * 512], F32R, kind="ExternalInput")
    # constants
    ones128 = nc.dram_tensor("ones128", [128, 128], F32R, kind="ExternalInput")  # 1/1024
    ident2 = nc.dram_tensor("ident2", [128, 64], F32, kind="ExternalInput")      # eye64 x2
    onescol = nc.dram_tensor("onescol", [128, 16], F32R, kind="ExternalInput")   # 1.0
    brow = nc.dram_tensor("brow", [1, 128], F32R, kind="ExternalInput")          # 1.0

    outT = nc.dram_tensor("outT", [128, 8 * 512], F32, kind="ExternalOutput")

    # collective bounce buffers
    ag_in = nc.dram_tensor("ag_in", [1, TOWN], F32)
    ag_out = nc.dram_tensor("ag_out", [NCORES, TOWN], F32, addr_space="Shared")
    a2a_in = [nc.dram_tensor(f"a2a_in{h}", [NCORES, 64, TOWN], mybir.dt.bfloat16)
              for h in range(2)]
    a2a_out = [nc.dram_tensor(f"a2a_out{h}", [NCORES, 64, TOWN], mybir.dt.bfloat16)
               for h in range(2)]

    dbg = {}
    if debug:
        for name, shape in [("dbg_q", [128, TOK]),
                            ("dbg_k", [128, TOK]), ("dbg_vtok", [128, 16 * 65]),
                            ("dbg_rs", [1, TOK]), ("dbg_pt", [128, 512]),
                            ("dbg_att", [128, 8 * 512]), ("dbg_x2", [128, 8 * 512]),
                            ("dbg_h1", [128, 4 * 512]), ("dbg_rs2", [1, TOWN]),
                            ("dbg_m2", [1, TOWN])]:
            dbg[name] = nc.dram_tensor(name, shape, F32, kind="ExternalOutput")

    RG = [list(range(NCORES))]

    def bc_ap(dram, nparts, ncols, offset=0):
        """partition-broadcast read AP over a DRAM row."""
        return bass_mod.AP(tensor=dram.ap().tensor, offset=offset,
                           ap=[[0, nparts], [1, ncols]])

    with tile.TileContext(nc) as tc:
        import contextlib
        with contextlib.ExitStack() as ctx:
            consts = ctx.enter_context(tc.tile_pool(name="consts", bufs=1))
            xrpool = ctx.enter_context(tc.tile_pool(name="xrpool", bufs=1))
            qkctx = contextlib.ExitStack()
            qkpool = qkctx.enter_context(tc.tile_pool(name="qk", bufs=1))
            vpool = qkctx.enter_context(tc.tile_pool(name="vtok", bufs=1))
            attsb = qkctx.enter_context(tc.tile_pool(name="attsb", bufs=2))
            attps = qkctx.enter_context(
                tc.tile_pool(name="attps", bufs=2, space="PSUM"))
            ph1ps_ctx = contextlib.ExitStack()
            ph1ps = ph1ps_ctx.enter_context(
                tc.tile_pool(name="ph1ps", bufs=2, space="PSUM"))

            ones_t = consts.tile([128, 128], F32R)
            nc.sync.dma_start(out=ones_t[:], in_=ones128[:, :])
            mask_t = consts.tile([128, 4, 512], F32R)
            nc.sync.dma_start(out=mask_t[:],
                              in_=maskt[:, :].rearrange("p (o q) -> p o q", o=4))
            eps_t = consts.tile([1, 1], F32)
            nc.vector.memset(eps_t[:], EPS)
            brow_t = consts.tile([1, 128], F32R)
            nc.sync.dma_start(out=brow_t[:], in_=brow[:, :])

            # persistent Q', K' (feature-major, 2 local heads x 64 dims)
            q_t = qkpool.tile([128, TOK], F32R)
            k_t = qkpool.tile([128, TOK], F32R)
            v_tok = [[vpool.tile([128, 16, 65], F32R, tag=f"vtok{b}{h}",
                                 name=f"vtok{b}{h}")
                      for h in range(2)] for b in range(B)]

            def attn_jt(hl, b, jt):
                o = hl * 64
                nkt = 4 * (jt + 1)
                po = attps.tile([65, 512], F32, tag="po", name="po", bufs=3)
                qs = q_t[o:o + 64, b * S + jt * 512: b * S + (jt + 1) * 512]
                for kt in range(nkt):
                    ps = attps.tile([128, 512], F32, tag="ps", name="ps")
                    ks = k_t[o:o + 64, b * S + kt * 128: b * S + (kt + 1) * 128]
                    nc.tensor.matmul(ps[:], ks, qs, start=True, stop=True)
                    pt = attsb.tile([128, 512], F32R, tag="pt", name="pt", bufs=4)
                    nc.scalar.activation(pt[:], ps[:], AF.Exp)
                    od = kt - 4 * jt
                    if od >= 0:
                        nc.vector.tensor_mul(pt[:], pt.bitcast(F32)[:],
                                             mask_t.bitcast(F32)[:, od, :])
                    if debug and b == 0 and hl == 0 and jt == 0 and kt == 0:
                        nc.sync.dma_start(out=dbg["dbg_pt"][:, :],
                                          in_=pt.bitcast(F32)[:])
                    nc.tensor.matmul(po[:], v_tok[b][hl][:, kt, :], pt[:],
                                     start=(kt == 0), stop=(kt == nkt - 1))
                rcp = attsb.tile([1, 512], F32R, tag="rcp", name="rcp")
                with nc.allow_low_precision(reason="recip bcast f32r"):
                    nc.vector.reciprocal(rcp[:], po[64:65, :])
                psb = attps.tile([64, 512], F32, tag="ps", name="psb")
                nc.tensor.matmul(psb[:], brow_t[0:1, 0:64], rcp[:],
                                 start=True, stop=True)
                rb = attsb.tile([64, 512], F32, tag="rb", name="rb")
                nc.vector.tensor_copy(rb[:], psb[:])
                ov = attsb.tile([64, 512], mybir.dt.bfloat16, tag="ov", name="ov")
                nc.vector.tensor_mul(ov[:], po[0:64, :], rb[:])
                nc.sync.dma_start(out=a2a_in[hl][b * 4 + jt, :, :], in_=ov[:])

            xown = xrpool.tile([128, 8, 512], F32R)
            # ================= phase 1a: LN1 stats + AllGather =================
            with tc.tile_pool(name="ph1a", bufs=1) as ph1a:
                for dt in range(8):
                    nc.sync.dma_start(out=xown[:, dt, :],
                                      in_=xres[:, dt * 512:(dt + 1) * 512])
                sq = ph1a.tile([128, 8, 512], F32R)
                nc.vector.tensor_mul(sq[:], xown.bitcast(F32)[:], xown.bitcast(F32)[:])
                mps = ph1ps.tile([128, 512], F32, tag="qkvps")
                sps = ph1ps.tile([128, 512], F32, tag="qkvps")
                for dt in range(8):
                    nc.tensor.matmul(mps[:], ones_t[:], xown[:, dt, :],
                                     start=(dt == 0), stop=(dt == 7))
                for dt in range(8):
                    nc.tensor.matmul(sps[:], ones_t[:], sq[:, dt, :],
                                     start=(dt == 0), stop=(dt == 7))
                mrow = ph1a.tile([1, 512], F32)
                nc.vector.tensor_copy(mrow[:], mps[0:1, :])
                msq = ph1a.tile([1, 512], F32)
                nc.vector.tensor_copy(msq[:], sps[0:1, :])
                var = ph1a.tile([1, 512], F32)
                nc.vector.tensor_mul(var[:], mrow[:], mrow[:])
                nc.vector.tensor_sub(var[:], msq[:], var[:])
                rsq = ph1a.tile([1, 512], F32)
                nc.scalar.activation(rsq[:], var[:], AF.Sqrt, bias=eps_t[:])
                rs_own = ph1a.tile([1, 512], F32)
                nc.vector.reciprocal(rs_own[:], rsq[:])
                nc.sync.dma_start(out=ag_in[:, :], in_=rs_own[:])
                if sim:
                    nc.sync.dma_start(out=ag_out[:, :],
                                      in_=bc_ap(ag_in, NCORES, TOWN))
                else:
                    nc.gpsimd.collective_compute(
                        "AllGather", mybir.AluOpType.bypass, replica_groups=RG,
                        ins=[ag_in.ap().opt()], outs=[ag_out.ap().opt()])
                if debug:
                    nc.sync.dma_start(
                        out=dbg["dbg_rs"][:, :],
                        in_=ag_out[:, :].rearrange("r t -> (r t)")
                        .rearrange("(o t) -> o t", o=1))

            # ====== phase 1b: qkv matmuls + RoPE + V fixup, per 512-token tile =
            with tc.tile_pool(name="ph1b", bufs=1) as ph1b, \
                 tc.tile_pool(name="ph1x", bufs=2) as ph1x:
                id_t = ph1b.tile([128, 64], F32)
                nc.sync.dma_start(out=id_t[:], in_=ident2[:, :])
                ocol_t = ph1b.tile([128, 16], F32R)
                nc.sync.dma_start(out=ocol_t[:], in_=onescol[:, :])
                cos_t = ph1b.tile([128, S], F32)
                nc.sync.dma_start(out=cos_t[:], in_=cosr[:, :])
                sin_t = ph1b.tile([128, S], F32)
                nc.sync.dma_start(out=sin_t[:], in_=sinr[:, :])
                rs_col = ph1b.tile([128, 32], F32)
                nc.sync.dma_start(
                    out=rs_col[:],
                    in_=bass_mod.AP(tensor=ag_out.ap().tensor, offset=0,
                                    ap=[[1, 128], [128, 32]]))
                wq_t = ph1b.tile([128, 8, 384], mybir.dt.bfloat16)
                nc.sync.dma_start(out=wq_t[:],
                                  in_=wqkv[:, :].rearrange("p (d c) -> p d c", d=8))
                for b in range(B):
                    for hl in range(2):
                        nc.sync.dma_start(
                            out=v_tok[b][hl][:, :, 64:65],
                            in_=ocol_t[:].rearrange("p (s o) -> p s o", o=1))

                for t in range(8):
                    b, soff = t // 4, (t % 4) * 512
                    tsl = slice(t * 512, (t + 1) * 512)
                    ssl = slice(soff, soff + 512)
                    xt = ph1x.tile([128, 8, 512], mybir.dt.bfloat16, tag="xt",
                                   bufs=3)
                    for dt in range(8):
                        nc.sync.dma_start(
                            out=xt[:, dt, :],
                            in_=xtp[:, t * 4096 + dt * 512: t * 4096 + (dt + 1) * 512])
                    v_raw = ph1x.tile([128, 512], F32, tag="vraw")
                    raws = [(q_t, None), (k_t, None), (None, v_raw)]
                    for jb in range(3):
                        pj = ph1ps.tile([128, 512], F32, tag="qkvps")
                        for dt in range(8):
                            nc.tensor.matmul(
                                pj[:], wq_t[:, dt, jb * 128:(jb + 1) * 128],
                                xt[:, dt, :], start=(dt == 0), stop=(dt == 7))
                        big, small = raws[jb]
                        if big is not None:
                            nc.vector.tensor_copy(big[:, tsl], pj[:])
                        else:
                            nc.vector.tensor_copy(small[:], pj[:])
                    # RoPE on this token tile (both heads in one op)
                    rs_c = ph1x.tile([128, 512], F32, tag="rsc")
                    nc.sync.dma_start(out=rs_c[:],
                                      in_=bc_ap(ag_out, 128, 512, offset=t * 512))
                    for tn in (q_t, k_t):
                        swp = ph1x.tile([128, 512], F32, tag="swp")
                        for hl in range(2):
                            o = hl * 64
                            nc.sync.dma_start(out=swp[o:o + 32, :],
                                              in_=tn.bitcast(F32)[o + 32:o + 64, tsl])
                            nc.sync.dma_start(out=swp[o + 32:o + 64, :],
                                              in_=tn.bitcast(F32)[o:o + 32, tsl])
                        ta = ph1x.tile([128, 512], F32, tag="ropea")
                        tb = ph1x.tile([128, 512], F32, tag="ropeb")
                        nc.vector.tensor_mul(ta[:], tn.bitcast(F32)[:, tsl],
                                             cos_t[:, ssl])
                        nc.vector.tensor_mul(tb[:], swp[:], sin_t[:, ssl])
                        nc.vector.tensor_add(ta[:], ta[:], tb[:])
                        nc.vector.tensor_mul(tn[:, tsl], ta[:], rs_c[:])
                    # V transposes for this tile (4 x 128 tokens, each head)
                    for hl in range(2):
                        for ktl in range(4):
                            kt = (t % 4) * 4 + ktl
                            tp = ph1ps.tile([128, 64], F32, tag="trps", bufs=1)
                            nc.tensor.transpose(
                                tp[:], v_raw[hl * 64:(hl + 1) * 64,
                                             ktl * 128:(ktl + 1) * 128],
                                id_t[hl * 64:(hl + 1) * 64, :])
                            nc.scalar.activation(
                                v_tok[b][hl][:, kt, 0:64], tp[:], AF.Copy,
                                scale=rs_col[:, b * 16 + kt: b * 16 + kt + 1])
                    attn_jt(0, b, t % 4)
                if debug:
                    nc.sync.dma_start(out=dbg["dbg_q"][:, :],
                                      in_=q_t.bitcast(F32)[:])
                    nc.sync.dma_start(out=dbg["dbg_k"][:, :],
                                      in_=k_t.bitcast(F32)[:])
                    nc.sync.dma_start(out=dbg["dbg_vtok"][:, :],
                                      in_=v_tok[0][0].bitcast(F32)[:]
                                      .rearrange("p a b -> p (a b)"))
            ph1ps_ctx.close()

            # ==== phase 2 remainder: second head attention + both AllToAlls ====
            if sim:
                nc.sync.dma_start(out=a2a_out[0].ap(), in_=a2a_in[0].ap())
            else:
                nc.gpsimd.collective_compute(
                    "AllToAll", mybir.AluOpType.bypass, replica_groups=RG,
                    ins=[a2a_in[0].ap().opt()], outs=[a2a_out[0].ap().opt()])
            for b in range(B):
                for jt in range(4):
                    attn_jt(1, b, jt)
            if sim:
                nc.sync.dma_start(out=a2a_out[1].ap(), in_=a2a_in[1].ap())
            else:
                nc.gpsimd.collective_compute(
                    "AllToAll", mybir.AluOpType.bypass, replica_groups=RG,
                    ins=[a2a_in[1].ap().opt()], outs=[a2a_out[1].ap().opt()])
            qkctx.close()  # free Q/K/V + attention SBUF/PSUM before the FFN phase

            # ============ phase 3: proj + residual + LN2 + FFN =================
            with tc.tile_pool(name="ph3", bufs=1) as ph3, \
                 tc.tile_pool(name="ph3w", bufs=2) as ph3w, \
                 tc.tile_pool(name="ph3ps", bufs=2, space="PSUM") as ph3ps:
                attn_t = ph3.tile([128, 8, 512], mybir.dt.bfloat16, tag="bigA")
                for hl in range(2):
                    nc.sync.dma_start(
                        out=attn_t[hl * 64:(hl + 1) * 64, :, :],
                        in_=a2a_out[hl].ap().rearrange("i f t -> f i t"))
                if debug:
                    atf = ph3w.tile([128, 8, 512], F32, tag="atf")
                    nc.vector.tensor_copy(atf[:], attn_t[:])
                    nc.sync.dma_start(out=dbg["dbg_att"][:, :],
                                      in_=atf[:].rearrange("p a b -> p (a b)"))
                x2 = ph3.tile([128, 8, 512], F32R)
                for jb in range(8):
                    wpc = ph3w.tile([128, 8, 128], mybir.dt.bfloat16, tag="wpc")
                    nc.sync.dma_start(
                        out=wpc[:], in_=wproj[:, jb * 1024:(jb + 1) * 1024]
                        .rearrange("p (d c) -> p d c", d=8))
                    pp = ph3ps.tile([128, 512], F32, tag="pp")
                    for dt in range(8):
                        nc.tensor.matmul(pp[:], wpc[:, dt, :], attn_t[:, dt, :],
                                         start=(dt == 0), stop=(dt == 7))
                    nc.vector.tensor_add(x2[:, jb, :], pp[:],
                                         xown.bitcast(F32)[:, jb, :])
                if debug:
                    nc.sync.dma_start(out=dbg["dbg_x2"][:, :],
                                      in_=x2.bitcast(F32)[:]
                                      .rearrange("p a b -> p (a b)"))

                # LN2 stats (local, own tokens)
                sq2 = ph3.tile([128, 8, 512], F32R, tag="bigB")
                nc.vector.tensor_mul(sq2[:], x2.bitcast(F32)[:], x2.bitcast(F32)[:])
                mps2 = ph3ps.tile([128, 512], F32, tag="mps2", bufs=1)
                sps2 = ph3ps.tile([128, 512], F32, tag="sps2", bufs=1)
                for dt in range(8):
                    nc.tensor.matmul(mps2[:], ones_t[:], x2[:, dt, :],
                                     start=(dt == 0), stop=(dt == 7))
                for dt in range(8):
                    nc.tensor.matmul(sps2[:], ones_t[:], sq2[:, dt, :],
                                     start=(dt == 0), stop=(dt == 7))
                m2 = ph3.tile([1, 512], F32)
                nc.vector.tensor_copy(m2[:], mps2[0:1, :])
                msq2 = ph3.tile([1, 512], F32)
                nc.vector.tensor_copy(msq2[:], sps2[0:1, :])
                var2 = ph3.tile([1, 512], F32)
                nc.vector.tensor_mul(var2[:], m2[:], m2[:])
                nc.vector.tensor_sub(var2[:], msq2[:], var2[:])
                rsq2 = ph3.tile([1, 512], F32)
                nc.scalar.activation(rsq2[:], var2[:], AF.Sqrt, bias=eps_t[:])
                rs2 = ph3.tile([1, 512], F32R)
                with nc.allow_low_precision(reason="rs2 bcast f32r"):
                    nc.vector.reciprocal(rs2[:], rsq2[:])
                if debug:
                    nc.sync.dma_start(out=dbg["dbg_rs2"][:, :],
                                      in_=rs2.bitcast(F32)[:])
                    nc.sync.dma_start(out=dbg["dbg_m2"][:, :], in_=m2[:])
                m2r = ph3.tile([1, 512], F32R)
                nc.vector.tensor_copy(m2r[:], m2[:])
                m2b = ph3ps.tile([128, 512], F32, tag="pp")
                nc.tensor.matmul(m2b[:], brow_t[:], m2r[:], start=True, stop=True)
                rs2b = ph3ps.tile([128, 512], F32, tag="pp")
                nc.tensor.matmul(rs2b[:], brow_t[:], rs2[:], start=True, stop=True)
                x2n = ph3.tile([128, 8, 512], mybir.dt.bfloat16)
                for dt in range(8):
                    tnrm = ph3w.tile([128, 512], F32, tag="tnrm")
                    nc.vector.tensor_sub(tnrm[:], x2.bitcast(F32)[:, dt, :], m2b[:])
                    nc.vector.tensor_mul(x2n[:, dt, :], tnrm[:], rs2b[:])

                # fc1 + gelu
                h1 = ph3.tile([128, 32, 512], mybir.dt.bfloat16)
                for jg in range(16):  # 16 chunks of 2 j-tiles
                    wc = ph3w.tile([128, 2048], mybir.dt.bfloat16, tag="wc")
                    nc.sync.dma_start(out=wc[:],
                                      in_=wfc1[:, jg * 2048:(jg + 1) * 2048])
                    for jj in range(2):
                        j = jg * 2 + jj
                        pf = ph3ps.tile([128, 512], F32, tag="pf")
                        for dt in range(8):
                            nc.tensor.matmul(
                                pf[:],
                                wc[:, (jj * 8 + dt) * 128:(jj * 8 + dt + 1) * 128],
                                x2n[:, dt, :], start=(dt == 0), stop=(dt == 7))
                        nc.scalar.activation(h1[:, j, :], pf[:], AF.Gelu)
                if debug:
                    h1f = ph3w.tile([128, 4, 512], F32, tag="h1f")
                    nc.vector.tensor_copy(h1f[:], h1[:, 0:4, :])
                    nc.sync.dma_start(out=dbg["dbg_h1"][:, :],
                                      in_=h1f[:].rearrange("p a b -> p (a b)"))

                # fc2 + residual + out
                for d in range(8):
                    pf2 = ph3ps.tile([128, 512], F32, tag="pf2")
                    for half in range(2):
                        wc2 = ph3w.tile([128, 2048], mybir.dt.bfloat16, tag="wc")
                        nc.sync.dma_start(
                            out=wc2[:],
                            in_=wfc2[:, (d * 32 + half * 16) * 128:
                                     (d * 32 + (half + 1) * 16) * 128])
                        for jj in range(16):
                            jt = half * 16 + jj
                            nc.tensor.matmul(pf2[:], wc2[:, jj * 128:(jj + 1) * 128],
                                             h1[:, jt, :], start=(jt == 0),
                                             stop=(jt == 31))
                    ot = ph3w.tile([128, 512], F32, tag="ot")
                    nc.vector.tensor_add(ot[:], pf2[:], x2.bitcast(F32)[:, d, :])
                    nc.sync.dma_start(out=outT[:, d * 512:(d + 1) * 512], in_=ot[:])

    nc.compile()
    _BUILD_CACHE[key] = nc
    return nc


def _prep_inputs(x, sin, cos, ln1_w, w_qkv, w_proj, ln2_w, w_fc1, w_fc2):
    """Host-side packing/folding. Returns in_maps (list of 8 dicts)."""
    import ml_dtypes
    xf = np.ascontiguousarray(x.reshape(TOK, D).T)          # [1024, 4096]
    xtp_r = round_fp32r(xf)
    xbf = xf.astype(ml_dtypes.bfloat16)
    xtp = np.empty((128, 8 * 8 * 512), ml_dtypes.bfloat16)
    for t in range(8):
        for dt in range(8):
            xtp[:, (t * 8 + dt) * 512:(t * 8 + dt + 1) * 512] = \
                xbf[dt * 128:(dt + 1) * 128, t * 512:(t + 1) * 512]

    # qkv weight fold: ln1_w scale, q-scale 1/8, mean-subtraction fold
    w1 = (ln1_w[:, None] * w_qkv).astype(np.float64)
    w1[:, :D] *= 1.0 / np.sqrt(HD)
    w1 = w1 - w1.mean(axis=0, keepdims=True)
    w1 = w1.astype(np.float32)

    wp = w_proj.astype(ml_dtypes.bfloat16)
    wproj_p = np.empty((128, 8 * 8 * 128), ml_dtypes.bfloat16)
    for jb in range(8):
        for dt in range(8):
            wproj_p[:, (jb * 8 + dt) * 128:(jb * 8 + dt + 1) * 128] = \
                wp[dt * 128:(dt + 1) * 128, jb * 128:(jb + 1) * 128]

    wf1 = (ln2_w[:, None] * w_fc1).astype(ml_dtypes.bfloat16)  # [1024, 4096]
    wfc1_p = np.empty((128, 32 * 8 * 128), ml_dtypes.bfloat16)
    for j in range(32):
        for dt in range(8):
            wfc1_p[:, (j * 8 + dt) * 128:(j * 8 + dt + 1) * 128] = \
                wf1[dt * 128:(dt + 1) * 128, j * 128:(j + 1) * 128]
    wf2 = w_fc2.astype(ml_dtypes.bfloat16)                   # [4096, 1024]
    wfc2_p = np.empty((128, 8 * 32 * 128), ml_dtypes.bfloat16)
    for d in range(8):
        for jt in range(32):
            wfc2_p[:, (d * 32 + jt) * 128:(d * 32 + jt + 1) * 128] = \
                wf2[jt * 128:(jt + 1) * 128, d * 128:(d + 1) * 128]

    cos1 = np.ascontiguousarray(cos.reshape(S, HD).T, dtype=np.float32)  # [64, S]
    sin1 = np.ascontiguousarray(sin.reshape(S, HD).T, dtype=np.float32)
    sin1[:HD // 2, :] *= -1.0
    cosf = np.concatenate([cos1, cos1], axis=0)  # [128, S]: both local heads
    sinT = np.concatenate([sin1, sin1], axis=0)

    maskt = np.zeros((128, 4, 512), np.float32)
    rk = np.arange(128)[:, None]
    rq = np.arange(512)[None, :]
    for o in range(4):
        maskt[:, o, :] = (128 * o + rk <= rq).astype(np.float32)
    maskt = maskt.reshape(128, 4 * 512)

    ones128 = np.full((128, 128), 1.0 / D, np.float32)  # 2^-10, fp32r exact
    brow = np.ones((1, 128), np.float32)
    ident2 = np.concatenate([np.eye(64, dtype=np.float32)] * 2, axis=0)
    onescol = np.ones((128, 16), np.float32)

    in_maps = []
    for c in range(NCORES):
        heads = [2 * c, 2 * c + 1]
        cols = []
        for blk in range(3):  # Q, K, V
            for h in heads:
                cols.extend(range(blk * D + h * HD, blk * D + (h + 1) * HD))
        wsel = w1[:, cols].astype(ml_dtypes.bfloat16)  # [1024, 384]
        wqkv_p = np.empty((128, 8 * 384), ml_dtypes.bfloat16)
        for dt in range(8):
            wqkv_p[:, dt * 384:(dt + 1) * 384] = wsel[dt * 128:(dt + 1) * 128, :]
        xres = np.empty((128, 8 * 512), np.float32)
        xslice = xtp_r[:, c * TOWN:(c + 1) * TOWN]  # [1024, 512] pre-rounded
        for dt in range(8):
            xres[:, dt * 512:(dt + 1) * 512] = xslice[dt * 128:(dt + 1) * 128, :]
        in_maps.append({
            "xtp": xtp, "xres": xres, "wqkv": wqkv_p, "wproj": wproj_p,
            "wfc1": wfc1_p, "wfc2": wfc2_p, "cosr": cosf, "sinr": sinT,
            "maskt": maskt, "ones128": ones128, "ident2": ident2,
            "onescol": onescol, "brow": brow,
        })
    return in_maps


def _assemble_output(results):
    full = np.empty((TOK, D), np.float32)
    for c in range(NCORES):
        blk = results[c]["outT"].reshape(128, 8, 512)
        for d in range(8):
            full[c * TOWN:(c + 1) * TOWN, d * 128:(d + 1) * 128] = blk[:, d, :].T
    return full.reshape(B, S, D)


def kernel(x, mask, sin, cos, ln1_w, ln1_b, w_qkv, w_proj, ln2_w, ln2_b,
           w_fc1, w_fc2):
    x = np.asarray(x, np.float32)
    mask_np = np.asarray(mask)
    causal = np.array_equal(
        mask_np.reshape(S, S), np.tril(np.ones((S, S), dtype=bool)))
    biases_zero = (np.abs(np.asarray(ln1_b)).max() == 0.0 and
                   np.abs(np.asarray(ln2_b)).max() == 0.0)
    if not (causal and biases_zero):
        return _np_reference(x, mask_np, np.asarray(sin), np.asarray(cos),
                             np.asarray(ln1_w), np.asarray(ln1_b),
                             np.asarray(w_qkv), np.asarray(w_proj),
                             np.asarray(ln2_w), np.asarray(ln2_b),
                             np.asarray(w_fc1), np.asarray(w_fc2))

    from concourse.bass_utils import run_bass_kernel_spmd
    nc = _build(debug=False)
    in_maps = _prep_inputs(x, np.asarray(sin, np.float32).reshape(S, HD),
                           np.asarray(cos, np.float32).reshape(S, HD),
                           np.asarray(ln1_w, np.float32),
                           np.asarray(w_qkv, np.float32),
                           np.asarray(w_proj, np.float32),
                           np.asarray(ln2_w, np.float32),
                           np.asarray(w_fc1, np.float32),
                           np.asarray(w_fc2, np.float32))
    res = run_bass_kernel_spmd(nc, in_maps, core_ids=list(range(NCORES)))
    return _assemble_output(res.results)


# revision 29
# speedup vs baseline: 1.0205x; 1.0205x over previous
"""Trainium2 Bass kernel for nn_Block_51178830299350 (dense transformer block).

Strategy (8 NeuronCores, single NEFF):
  - Head-tensor-parallel attention: 2 heads/core over all 4096 tokens.
  - LN1: per-core stats on own 512 tokens + tiny AllGather of rsqrt row.
    Mean-subtraction folded into pre-packed qkv weights (host side).
  - fp32r (TF32-like) matmuls everywhere: full PE rate, ~1e-4 rel err.
  - Flash-style transposed scores [k, q]; softmax denominator via a ones
    column appended to V; causal handled by loop bounds + 0/1 diagonal masks.
  - AllToAll (2MB/rank) to redistribute head-sharded attention output to
    token-sharded; proj + FFN run token-parallel with full weights streamed.
"""
import sys

sys.path.insert(0, "/opt/trn_rl_repo")

import numpy as np

B, S, D, H, HD = 2, 2048, 1024, 16, 64
INNER = 4 * D
NCORES = 8
TOK = B * S              # 4096 flat tokens
TOWN = TOK // NCORES     # 512 tokens per core
EPS = 1e-5
_BUILD_CACHE = {}


def round_fp32r(x: np.ndarray) -> np.ndarray:
    """Round fp32 -> fp32r (11-bit mantissa, RTNE), matching TRN2 PE input fmt."""
    v = np.ascontiguousarray(x, dtype=np.float32).view(np.uint32)
    low = v & np.uint32(0xFFF)
    half = np.uint32(0x800)
    rounded = (v & ~np.uint32(0xFFF)).copy()
    up = (low > half) | ((low == half) & (((v >> np.uint32(12)) & np.uint32(1)) != 0))
    rounded[up] += np.uint32(0x1000)
    return rounded.view(np.float32)


def _np_reference(x, mask, sin, cos, ln1_w, ln1_b, w_qkv, w_proj, ln2_w, ln2_b,
                  w_fc1, w_fc2):
    """Slow numpy fallback (only used if inputs violate kernel assumptions)."""
    from scipy.special import erf

    def ln(t, w, b):
        m = t.mean(-1, keepdims=True)
        v = ((t - m) ** 2).mean(-1, keepdims=True)
        return (t - m) / np.sqrt(v + EPS) * w + b

    def rope(t, sin, cos):
        half = t.shape[-1] // 2
        rot = np.concatenate([-t[..., half:], t[..., :half]], -1)
        return t * cos + rot * sin

    b, s, d = x.shape
    hx = ln(x, ln1_w, ln1_b)
    qkv = (hx @ w_qkv).reshape(b, s, 3, H, HD).transpose(2, 0, 3, 1, 4)
    q, k, v = qkv[0], qkv[1], qkv[2]
    q = rope(q, sin, cos)
    k = rope(k, sin, cos)
    att = np.einsum("bhqd,bhkd->bhqk", q, k) / np.sqrt(HD)
    att = np.where(mask, att, -np.inf)
    att = att - att.max(-1, keepdims=True)
    p = np.exp(att)
    p /= p.sum(-1, keepdims=True)
    o = np.einsum("bhqk,bhkd->bhqd", p, v)
    o = o.transpose(0, 2, 1, 3).reshape(b, s, d) @ w_proj
    x = x + o
    h2 = ln(x, ln2_w, ln2_b)
    h2 = h2 @ w_fc1
    h2 = 0.5 * h2 * (1.0 + erf(h2 / np.sqrt(2.0)))
    h2 = h2 @ w_fc2
    return (x + h2).astype(np.float32)


def _build(debug=False, sim=False):
    key = ("nc", debug, sim)
    if key in _BUILD_CACHE:
        return _BUILD_CACHE[key]
    import concourse.bacc as bacc
    import concourse.bass as bass_mod
    import concourse.tile as tile
    from concourse import mybir

    F32 = mybir.dt.float32
    F32R = mybir.dt.float32r
    AF = mybir.ActivationFunctionType

    nc = bacc.Bacc("TRN2", target_bir_lowering=False, debug=False,
                   enable_asserts=False, num_devices=NCORES)

    # ---------------- DRAM parameters (per core) ----------------
    # packed transposed x: [128, (t*8+dt)*512]; (p, t, dt, i) = xT[dt*128+p, t*512+i]
    xtp = nc.dram_tensor("xtp", [128, 8 * 8 * 512], mybir.dt.bfloat16,
                         kind="ExternalInput")
    # own-slice xT (pre-rounded) for stats + residual: [128, dt*512 + i]
    xres = nc.dram_tensor("xres", [128, 8 * 512], F32R, kind="ExternalInput")
    # qkv weights for this core's 2 heads: [128, dt*384 + {Q(128)|K(128)|V(128)}]
    wqkv = nc.dram_tensor("wqkv", [128, 8 * 384], mybir.dt.bfloat16,
                          kind="ExternalInput")
    # proj weights (full): [128, (jb*8+dt)*128 + j]
    wproj = nc.dram_tensor("wproj", [128, 8 * 8 * 128], mybir.dt.bfloat16, kind="ExternalInput")
    # fc1 (full, ln2_w folded): [128, (j*8+dt)*128 + jj]
    wfc1 = nc.dram_tensor("wfc1", [128, 32 * 8 * 128], mybir.dt.bfloat16, kind="ExternalInput")
    # fc2 (full): [128, (d*32+jt)*128 + dd]
    wfc2 = nc.dram_tensor("wfc2", [128, 8 * 32 * 128], mybir.dt.bfloat16, kind="ExternalInput")
    # rope tables [128, S]: head-dim table stacked twice (both local heads),
    # transposed, sign-folded sin (batch-independent)
    cosr = nc.dram_tensor("cosr", [128, S], F32, kind="ExternalInput")
    sinr = nc.dram_tensor("sinr", [128, S], F32, kind="ExternalInput")
    # 4 canonical 0/1 diagonal mask tiles [128k, 512q]
    maskt = nc.dram_tensor("maskt", [128, 4 * 512], F32R, kind="ExternalInput")
    # constants
    ones128 = nc.dram_tensor("ones128", [128, 128], F32R, kind="ExternalInput")  # 1/1024
    ident2 = nc.dram_tensor("ident2", [128, 64], F32, kind="ExternalInput")      # eye64 x2
    onescol = nc.dram_tensor("onescol", [128, 16], F32R, kind="ExternalInput")   # 1.0
    brow = nc.dram_tensor("brow", [1, 128], F32R, kind="ExternalInput")          # 1.0

    outT = nc.dram_tensor("outT", [128, 8 * 512], F32, kind="ExternalOutput")

    # collective bounce buffers
    ag_in = nc.dram_tensor("ag_in", [1, TOWN], F32)
    ag_out = nc.dram_tensor("ag_out", [NCORES, TOWN], F32, addr_space="Shared")
    a2a_in = [nc.dram_tensor(f"a2a_in{h}", [NCORES, 64, TOWN], mybir.dt.bfloat16)
              for h in range(2)]
    a2a_out = [nc.dram_tensor(f"a2a_out{h}", [NCORES, 64, TOWN], mybir.dt.bfloat16)
               for h in range(2)]

    dbg = {}
    if debug:
        for name, shape in [("dbg_q", [128, TOK]),
                            ("dbg_k", [128, TOK]), ("dbg_vtok", [128, 16 * 65]),
                            ("dbg_rs", [1, TOK]), ("dbg_pt", [128, 512]),
                            ("dbg_att", [128, 8 * 512]), ("dbg_x2", [128, 8 * 512]),
                            ("dbg_h1", [128, 4 * 512]), ("dbg_rs2", [1, TOWN]),
                            ("dbg_m2", [1, TOWN])]:
            dbg[name] = nc.dram_tensor(name, shape, F32, kind="ExternalOutput")

    RG = [list(range(NCORES))]

    def bc_ap(dram, nparts, ncols, offset=0):
        """partition-broadcast read AP over a DRAM row."""
        return bass_mod.AP(tensor=dram.ap().tensor, offset=offset,
                           ap=[[0, nparts], [1, ncols]])

    with tile.TileContext(nc) as tc:
        import contextlib
        with contextlib.ExitStack() as ctx:
            consts = ctx.enter_context(tc.tile_pool(name="consts", bufs=1))
            xrpool = ctx.enter_context(tc.tile_pool(name="xrpool", bufs=1))
            qkctx = contextlib.ExitStack()
            qkpool = qkctx.enter_context(tc.tile_pool(name="qk", bufs=1))
            vpool = qkctx.enter_context(tc.tile_pool(name="vtok", bufs=1))
            attsb = qkctx.enter_context(tc.tile_pool(name="attsb", bufs=2))
            attps = qkctx.enter_context(
                tc.tile_pool(name="attps", bufs=2, space="PSUM"))
            ph1ps_ctx = contextlib.ExitStack()
            ph1ps = ph1ps_ctx.enter_context(
                tc.tile_pool(name="ph1ps", bufs=2, space="PSUM"))

            ones_t = consts.tile([128, 128], F32R)
            nc.sync.dma_start(out=ones_t[:], in_=ones128[:, :])
            mask_t = consts.tile([128, 4, 512], F32R)
            nc.sync.dma_start(out=mask_t[:],
                              in_=maskt[:, :].rearrange("p (o q) -> p o q", o=4))
            eps_t = consts.tile([1, 1], F32)
            nc.vector.memset(eps_t[:], EPS)
            brow_t = consts.tile([1, 128], F32R)
            nc.sync.dma_start(out=brow_t[:], in_=brow[:, :])

            # persistent Q', K' (feature-major, 2 local heads x 64 dims)
            q_t = qkpool.tile([128, TOK], F32R)
            k_t = qkpool.tile([128, TOK], F32R)
            v_tok = [[vpool.tile([128, 16, 65], F32R, tag=f"vtok{b}{h}",
                                 name=f"vtok{b}{h}")
                      for h in range(2)] for b in range(B)]

            def attn_jt(hl, b, jt):
                o = hl * 64
                nkt = 4 * (jt + 1)
                po = attps.tile([65, 512], F32, tag="po", name="po", bufs=3)
                qs = q_t[o:o + 64, b * S + jt * 512: b * S + (jt + 1) * 512]
                for kt in range(nkt):
                    ps = attps.tile([128, 512], F32, tag="ps", name="ps")
                    ks = k_t[o:o + 64, b * S + kt * 128: b * S + (kt + 1) * 128]
                    nc.tensor.matmul(ps[:], ks, qs, start=True, stop=True)
                    pt = attsb.tile([128, 512], F32R, tag="pt", name="pt", bufs=4)
                    nc.scalar.activation(pt[:], ps[:], AF.Exp)
                    od = kt - 4 * jt
                    if od >= 0:
                        nc.vector.tensor_mul(pt[:], pt.bitcast(F32)[:],
                                             mask_t.bitcast(F32)[:, od, :])
                    if debug and b == 0 and hl == 0 and jt == 0 and kt == 0:
                        nc.sync.dma_start(out=dbg["dbg_pt"][:, :],
                                          in_=pt.bitcast(F32)[:])
                    nc.tensor.matmul(po[:], v_tok[b][hl][:, kt, :], pt[:],
                                     start=(kt == 0), stop=(kt == nkt - 1))
                rcp = attsb.tile([1, 512], F32R, tag="rcp", name="rcp")
                with nc.allow_low_precision(reason="recip bcast f32r"):
                    nc.vector.reciprocal(rcp[:], po[64:65, :])
                psb = attps.tile([64, 512], F32, tag="ps", name="psb")
                nc.tensor.matmul(psb[:], brow_t[0:1, 0:64], rcp[:],
                                 start=True, stop=True)
                rb = attsb.tile([64, 512], F32, tag="rb", name="rb")
                nc.vector.tensor_copy(rb[:], psb[:])
                ov = attsb.tile([64, 512], mybir.dt.bfloat16, tag="ov", name="ov")
                nc.vector.tensor_mul(ov[:], po[0:64, :], rb[:])
                nc.sync.dma_start(out=a2a_in[hl][b * 4 + jt, :, :], in_=ov[:])

            xown = xrpool.tile([128, 8, 512], F32R)
            # ================= phase 1a: LN1 stats + AllGather =================
            with tc.tile_pool(name="ph1a", bufs=1) as ph1a:
                for dt in range(8):
                    nc.sync.dma_start(out=xown[:, dt, :],
                                      in_=xres[:, dt * 512:(dt + 1) * 512])
                sq = ph1a.tile([128, 8, 512], F32R)
                nc.vector.tensor_mul(sq[:], xown.bitcast(F32)[:], xown.bitcast(F32)[:])
                mps = ph1ps.tile([128, 512], F32, tag="qkvps")
                sps = ph1ps.tile([128, 512], F32, tag="qkvps")
                for dt in range(8):
                    nc.tensor.matmul(mps[:], ones_t[:], xown[:, dt, :],
                                     start=(dt == 0), stop=(dt == 7))
                for dt in range(8):
                    nc.tensor.matmul(sps[:], ones_t[:], sq[:, dt, :],
                                     start=(dt == 0), stop=(dt == 7))
                mrow = ph1a.tile([1, 512], F32)
                nc.vector.tensor_copy(mrow[:], mps[0:1, :])
                msq = ph1a.tile([1, 512], F32)
                nc.vector.tensor_copy(msq[:], sps[0:1, :])
                var = ph1a.tile([1, 512], F32)
                nc.vector.tensor_mul(var[:], mrow[:], mrow[:])
                nc.vector.tensor_sub(var[:], msq[:], var[:])
                rsq = ph1a.tile([1, 512], F32)
                nc.scalar.activation(rsq[:], var[:], AF.Sqrt, bias=eps_t[:])
                rs_own = ph1a.tile([1, 512], F32)
                nc.vector.reciprocal(rs_own[:], rsq[:])
                nc.sync.dma_start(out=ag_in[:, :], in_=rs_own[:])
                if sim:
                    nc.sync.dma_start(out=ag_out[:, :],
                                      in_=bc_ap(ag_in, NCORES, TOWN))
                else:
                    nc.gpsimd.collective_compute(
                        "AllGather", mybir.AluOpType.bypass, replica_groups=RG,
                        ins=[ag_in.ap().opt()], outs=[ag_out.ap().opt()])
                if debug:
                    nc.sync.dma_start(
                        out=dbg["dbg_rs"][:, :],
                        in_=ag_out[:, :].rearrange("r t -> (r t)")
                        .rearrange("(o t) -> o t", o=1))

            # ====== phase 1b: qkv matmuls + RoPE + V fixup, per 512-token tile =
            with tc.tile_pool(name="ph1b", bufs=1) as ph1b, \
                 tc.tile_pool(name="ph1x", bufs=2) as ph1x:
                id_t = ph1b.tile([128, 64], F32)
                nc.sync.dma_start(out=id_t[:], in_=ident2[:, :])
                ocol_t = ph1b.tile([128, 16], F32R)
                nc.sync.dma_start(out=ocol_t[:], in_=onescol[:, :])
                cos_t = ph1b.tile([128, S], F32)
                nc.sync.dma_start(out=cos_t[:], in_=cosr[:, :])
                sin_t = ph1b.tile([128, S], F32)
                nc.sync.dma_start(out=sin_t[:], in_=sinr[:, :])
                rs_col = ph1b.tile([128, 32], F32)
                nc.sync.dma_start(
                    out=rs_col[:],
                    in_=bass_mod.AP(tensor=ag_out.ap().tensor, offset=0,
                                    ap=[[1, 128], [128, 32]]))
                wq_t = ph1b.tile([128, 8, 384], mybir.dt.bfloat16)
                nc.sync.dma_start(out=wq_t[:],
                                  in_=wqkv[:, :].rearrange("p (d c) -> p d c", d=8))
                for b in range(B):
                    for hl in range(2):
                        nc.sync.dma_start(
                            out=v_tok[b][hl][:, :, 64:65],
                            in_=ocol_t[:].rearrange("p (s o) -> p s o", o=1))

                for t in range(8):
                    b, soff = t // 4, (t % 4) * 512
                    tsl = slice(t * 512, (t + 1) * 512)
                    ssl = slice(soff, soff + 512)
                    xt = ph1x.tile([128, 8, 512], mybir.dt.bfloat16, tag="xt",
                                   bufs=4)
                    for dt in range(8):
                        nc.sync.dma_start(
                            out=xt[:, dt, :],
                            in_=xtp[:, t * 4096 + dt * 512: t * 4096 + (dt + 1) * 512])
                    v_raw = ph1x.tile([128, 512], F32, tag="vraw", bufs=3)
                    raws = [(q_t, None), (k_t, None), (None, v_raw)]
                    for jb in range(3):
                        pj = ph1ps.tile([128, 512], F32, tag="qkvps")
                        for dt in range(8):
                            nc.tensor.matmul(
                                pj[:], wq_t[:, dt, jb * 128:(jb + 1) * 128],
                                xt[:, dt, :], start=(dt == 0), stop=(dt == 7))
                        big, small = raws[jb]
                        if big is not None:
                            nc.vector.tensor_copy(big[:, tsl], pj[:])
                        else:
                            nc.vector.tensor_copy(small[:], pj[:])
                    # RoPE on this token tile (both heads in one op)
                    rs_c = ph1x.tile([128, 512], F32, tag="rsc", bufs=3)
                    nc.sync.dma_start(out=rs_c[:],
                                      in_=bc_ap(ag_out, 128, 512, offset=t * 512))
                    for tn in (q_t, k_t):
                        swp = ph1x.tile([128, 512], F32, tag="swp", bufs=3)
                        for hl in range(2):
                            o = hl * 64
                            nc.sync.dma_start(out=swp[o:o + 32, :],
                                              in_=tn.bitcast(F32)[o + 32:o + 64, tsl])
                            nc.sync.dma_start(out=swp[o + 32:o + 64, :],
                                              in_=tn.bitcast(F32)[o:o + 32, tsl])
                        ta = ph1x.tile([128, 512], F32, tag="ropea")
                        tb = ph1x.tile([128, 512], F32, tag="ropeb")
                        nc.vector.tensor_mul(ta[:], tn.bitcast(F32)[:, tsl],
                                             cos_t[:, ssl])
                        nc.vector.tensor_mul(tb[:], swp[:], sin_t[:, ssl])
                        nc.vector.tensor_add(ta[:], ta[:], tb[:])
                        nc.vector.tensor_mul(tn[:, tsl], ta[:], rs_c[:])
                    # V transposes for this tile (4 x 128 tokens, each head)
                    for hl in range(2):
                        for ktl in range(4):
                            kt = (t % 4) * 4 + ktl
                            tp = ph1ps.tile([128, 64], F32, tag="trps", bufs=1)
                            nc.tensor.transpose(
                                tp[:], v_raw[hl * 64:(hl + 1) * 64,
                                             ktl * 128:(ktl + 1) * 128],
                                id_t[hl * 64:(hl + 1) * 64, :])
                            nc.scalar.activation(
                                v_tok[b][hl][:, kt, 0:64], tp[:], AF.Copy,
                                scale=rs_col[:, b * 16 + kt: b * 16 + kt + 1])
                    attn_jt(0, b, t % 4)
                if debug:
                    nc.sync.dma_start(out=dbg["dbg_q"][:, :],
                                      in_=q_t.bitcast(F32)[:])
                    nc.sync.dma_start(out=dbg["dbg_k"][:, :],
                                      in_=k_t.bitcast(F32)[:])
                    nc.sync.dma_start(out=dbg["dbg_vtok"][:, :],
                                      in_=v_tok[0][0].bitcast(F32)[:]
                                      .rearrange("p a b -> p (a b)"))
            ph1ps_ctx.close()

            # ==== phase 2 remainder: second head attention + both AllToAlls ====
            if sim:
                nc.sync.dma_start(out=a2a_out[0].ap(), in_=a2a_in[0].ap())
            else:
                nc.gpsimd.collective_compute(
                    "AllToAll", mybir.AluOpType.bypass, replica_groups=RG,
                    ins=[a2a_in[0].ap().opt()], outs=[a2a_out[0].ap().opt()])
            for b in range(B):
                for jt in range(4):
                    attn_jt(1, b, jt)
            if sim:
                nc.sync.dma_start(out=a2a_out[1].ap(), in_=a2a_in[1].ap())
            else:
                nc.gpsimd.collective_compute(
                    "AllToAll", mybir.AluOpType.bypass, replica_groups=RG,
                    ins=[a2a_in[1].ap().opt()], outs=[a2a_out[1].ap().opt()])
            qkctx.close()  # free Q/K/V + attention SBUF/PSUM before the FFN phase

            # ============ phase 3: proj + residual + LN2 + FFN =================
            with tc.tile_pool(name="ph3", bufs=1) as ph3, \
                 tc.tile_pool(name="ph3w", bufs=2) as ph3w, \
                 tc.tile_pool(name="ph3ps", bufs=2, space="PSUM") as ph3ps:
                attn_t = ph3.tile([128, 8, 512], mybir.dt.bfloat16, tag="bigA")
                for hl in range(2):
                    nc.sync.dma_start(
                        out=attn_t[hl * 64:(hl + 1) * 64, :, :],
                        in_=a2a_out[hl].ap().rearrange("i f t -> f i t"))
                if debug:
                    atf = ph3w.tile([128, 8, 512], F32, tag="atf")
                    nc.vector.tensor_copy(atf[:], attn_t[:])
                    nc.sync.dma_start(out=dbg["dbg_att"][:, :],
                                      in_=atf[:].rearrange("p a b -> p (a b)"))
                x2 = ph3.tile([128, 8, 512], F32R)
                for jb in range(8):
                    wpc = ph3w.tile([128, 8, 128], mybir.dt.bfloat16, tag="wpc")
                    nc.sync.dma_start(
                        out=wpc[:], in_=wproj[:, jb * 1024:(jb + 1) * 1024]
                        .rearrange("p (d c) -> p d c", d=8))
                    pp = ph3ps.tile([128, 512], F32, tag="pp")
                    for dt in range(8):
                        nc.tensor.matmul(pp[:], wpc[:, dt, :], attn_t[:, dt, :],
                                         start=(dt == 0), stop=(dt == 7))
                    nc.vector.tensor_add(x2[:, jb, :], pp[:],
                                         xown.bitcast(F32)[:, jb, :])
                if debug:
                    nc.sync.dma_start(out=dbg["dbg_x2"][:, :],
                                      in_=x2.bitcast(F32)[:]
                                      .rearrange("p a b -> p (a b)"))

                # LN2 stats (local, own tokens)
                sq2 = ph3.tile([128, 8, 512], F32R, tag="bigB")
                nc.vector.tensor_mul(sq2[:], x2.bitcast(F32)[:], x2.bitcast(F32)[:])
                mps2 = ph3ps.tile([128, 512], F32, tag="mps2", bufs=1)
                sps2 = ph3ps.tile([128, 512], F32, tag="sps2", bufs=1)
                for dt in range(8):
                    nc.tensor.matmul(mps2[:], ones_t[:], x2[:, dt, :],
                                     start=(dt == 0), stop=(dt == 7))
                for dt in range(8):
                    nc.tensor.matmul(sps2[:], ones_t[:], sq2[:, dt, :],
                                     start=(dt == 0), stop=(dt == 7))
                m2 = ph3.tile([1, 512], F32)
                nc.vector.tensor_copy(m2[:], mps2[0:1, :])
                msq2 = ph3.tile([1, 512], F32)
                nc.vector.tensor_copy(msq2[:], sps2[0:1, :])
                var2 = ph3.tile([1, 512], F32)
                nc.vector.tensor_mul(var2[:], m2[:], m2[:])
                nc.vector.tensor_sub(var2[:], msq2[:], var2[:])
                rsq2 = ph3.tile([1, 512], F32)
                nc.scalar.activation(rsq2[:], var2[:], AF.Sqrt, bias=eps_t[:])
                rs2 = ph3.tile([1, 512], F32R)
                with nc.allow_low_precision(reason="rs2 bcast f32r"):
                    nc.vector.reciprocal(rs2[:], rsq2[:])
                if debug:
                    nc.sync.dma_start(out=dbg["dbg_rs2"][:, :],
                                      in_=rs2.bitcast(F32)[:])
                    nc.sync.dma_start(out=dbg["dbg_m2"][:, :], in_=m2[:])
                m2r = ph3.tile([1, 512], F32R)
                nc.vector.tensor_copy(m2r[:], m2[:])
                m2b = ph3ps.tile([128, 512], F32, tag="pp")
                nc.tensor.matmul(m2b[:], brow_t[:], m2r[:], start=True, stop=True)
                rs2b = ph3ps.tile([128, 512], F32, tag="pp")
                nc.tensor.matmul(rs2b[:], brow_t[:], rs2[:], start=True, stop=True)
                x2n = ph3.tile([128, 8, 512], mybir.dt.bfloat16)
                for dt in range(8):
                    tnrm = ph3w.tile([128, 512], F32, tag="tnrm")
                    nc.vector.tensor_sub(tnrm[:], x2.bitcast(F32)[:, dt, :], m2b[:])
                    nc.vector.tensor_mul(x2n[:, dt, :], tnrm[:], rs2b[:])

                # fc1 + gelu
                h1 = ph3.tile([128, 32, 512], mybir.dt.bfloat16)
                for jg in range(16):  # 16 chunks of 2 j-tiles
                    wc = ph3w.tile([128, 2048], mybir.dt.bfloat16, tag="wc")
                    nc.sync.dma_start(out=wc[:],
                                      in_=wfc1[:, jg * 2048:(jg + 1) * 2048])
                    for jj in range(2):
                        j = jg * 2 + jj
                        pf = ph3ps.tile([128, 512], F32, tag="pf")
                        for dt in range(8):
                            nc.tensor.matmul(
                                pf[:],
                                wc[:, (jj * 8 + dt) * 128:(jj * 8 + dt + 1) * 128],
                                x2n[:, dt, :], start=(dt == 0), stop=(dt == 7))
                        nc.scalar.activation(h1[:, j, :], pf[:], AF.Gelu)
                if debug:
                    h1f = ph3w.tile([128, 4, 512], F32, tag="h1f")
                    nc.vector.tensor_copy(h1f[:], h1[:, 0:4, :])
                    nc.sync.dma_start(out=dbg["dbg_h1"][:, :],
                                      in_=h1f[:].rearrange("p a b -> p (a b)"))

                # fc2 + residual + out
                for d in range(8):
                    pf2 = ph3ps.tile([128, 512], F32, tag="pf2")
                    for half in range(2):
                        wc2 = ph3w.tile([128, 2048], mybir.dt.bfloat16, tag="wc")
                        nc.sync.dma_start(
                            out=wc2[:],
                            in_=wfc2[:, (d * 32 + half * 16) * 128:
                                     (d * 32 + (half + 1) * 16) * 128])
                        for jj in range(16):
                            jt = half * 16 + jj
                            nc.tensor.matmul(pf2[:], wc2[:, jj * 128:(jj + 1) * 128],
                                             h1[:, jt, :], start=(jt == 0),
                                             stop=(jt == 31))
                    ot = ph3w.tile([128, 512], F32, tag="ot")
                    nc.vector.tensor_add(ot[:], pf2[:], x2.bitcast(F32)[:, d, :])
                    nc.sync.dma_start(out=outT[:, d * 512:(d + 1) * 512], in_=ot[:])

    nc.compile()
    _BUILD_CACHE[key] = nc
    return nc


def _prep_inputs(x, sin, cos, ln1_w, w_qkv, w_proj, ln2_w, w_fc1, w_fc2):
    """Host-side packing/folding. Returns in_maps (list of 8 dicts)."""
    import ml_dtypes
    xf = np.ascontiguousarray(x.reshape(TOK, D).T)          # [1024, 4096]
    xtp_r = round_fp32r(xf)
    xbf = xf.astype(ml_dtypes.bfloat16)
    xtp = np.empty((128, 8 * 8 * 512), ml_dtypes.bfloat16)
    for t in range(8):
        for dt in range(8):
            xtp[:, (t * 8 + dt) * 512:(t * 8 + dt + 1) * 512] = \
                xbf[dt * 128:(dt + 1) * 128, t * 512:(t + 1) * 512]

    # qkv weight fold: ln1_w scale, q-scale 1/8, mean-subtraction fold
    w1 = (ln1_w[:, None] * w_qkv).astype(np.float64)
    w1[:, :D] *= 1.0 / np.sqrt(HD)
    w1 = w1 - w1.mean(axis=0, keepdims=True)
    w1 = w1.astype(np.float32)

    wp = w_proj.astype(ml_dtypes.bfloat16)
    wproj_p = np.empty((128, 8 * 8 * 128), ml_dtypes.bfloat16)
    for jb in range(8):
        for dt in range(8):
            wproj_p[:, (jb * 8 + dt) * 128:(jb * 8 + dt + 1) * 128] = \
                wp[dt * 128:(dt + 1) * 128, jb * 128:(jb + 1) * 128]

    wf1 = (ln2_w[:, None] * w_fc1).astype(ml_dtypes.bfloat16)  # [1024, 4096]
    wfc1_p = np.empty((128, 32 * 8 * 128), ml_dtypes.bfloat16)
    for j in range(32):
        for dt in range(8):
            wfc1_p[:, (j * 8 + dt) * 128:(j * 8 + dt + 1) * 128] = \
                wf1[dt * 128:(dt + 1) * 128, j * 128:(j + 1) * 128]
    wf2 = w_fc2.astype(ml_dtypes.bfloat16)                   # [4096, 1024]
    wfc2_p = np.empty((128, 8 * 32 * 128), ml_dtypes.bfloat16)
    for d in range(8):
        for jt in range(32):
            wfc2_p[:, (d * 32 + jt) * 128:(d * 32 + jt + 1) * 128] = \
                wf2[jt * 128:(jt + 1) * 128, d * 128:(d + 1) * 128]

    cos1 = np.ascontiguousarray(cos.reshape(S, HD).T, dtype=np.float32)  # [64, S]
    sin1 = np.ascontiguousarray(sin.reshape(S, HD).T, dtype=np.float32)
    sin1[:HD // 2, :] *= -1.0
    cosf = np.concatenate([cos1, cos1], axis=0)  # [128, S]: both local heads
    sinT = np.concatenate([sin1, sin1], axis=0)

    maskt = np.zeros((128, 4, 512), np.float32)
    rk = np.arange(128)[:, None]
    rq = np.arange(512)[None, :]
    for o in range(4):
        maskt[:, o, :] = (128 * o + rk <= rq).astype(np.float32)
    maskt = maskt.reshape(128, 4 * 512)

    ones128 = np.full((128, 128), 1.0 / D, np.float32)  # 2^-10, fp32r exact
    brow = np.ones((1, 128), np.float32)
    ident2 = np.concatenate([np.eye(64, dtype=np.float32)] * 2, axis=0)
    onescol = np.ones((128, 16), np.float32)

    in_maps = []
    for c in range(NCORES):
        heads = [2 * c, 2 * c + 1]
        cols = []
        for blk in range(3):  # Q, K, V
            for h in heads:
                cols.extend(range(blk * D + h * HD, blk * D + (h + 1) * HD))
        wsel = w1[:, cols].astype(ml_dtypes.bfloat16)  # [1024, 384]
        wqkv_p = np.empty((128, 8 * 384), ml_dtypes.bfloat16)
        for dt in range(8):
            wqkv_p[:, dt * 384:(dt + 1) * 384] = wsel[dt * 128:(dt + 1) * 128, :]
        xres = np.empty((128, 8 * 512), np.float32)
        xslice = xtp_r[:, c * TOWN:(c + 1) * TOWN]  # [1024, 512] pre-rounded
        for dt in range(8):
            xres[:, dt * 512:(dt + 1) * 512] = xslice[dt * 128:(dt + 1) * 128, :]
        in_maps.append({
            "xtp": xtp, "xres": xres, "wqkv": wqkv_p, "wproj": wproj_p,
            "wfc1": wfc1_p, "wfc2": wfc2_p, "cosr": cosf, "sinr": sinT,
            "maskt": maskt, "ones128": ones128, "ident2": ident2,
            "onescol": onescol, "brow": brow,
        })
    return in_maps


def _assemble_output(results):
    full = np.empty((TOK, D), np.float32)
    for c in range(NCORES):
        blk = results[c]["outT"].reshape(128, 8, 512)
        for d in range(8):
            full[c * TOWN:(c + 1) * TOWN, d * 128:(d + 1) * 128] = blk[:, d, :].T
    return full.reshape(B, S, D)


def kernel(x, mask, sin, cos, ln1_w, ln1_b, w_qkv, w_proj, ln2_w, ln2_b,
           w_fc1, w_fc2):
    x = np.asarray(x, np.float32)
    mask_np = np.asarray(mask)
    causal = np.array_equal(
        mask_np.reshape(S, S), np.tril(np.ones((S, S), dtype=bool)))
    biases_zero = (np.abs(np.asarray(ln1_b)).max() == 0.0 and
                   np.abs(np.asarray(ln2_b)).max() == 0.0)
    if not (causal and biases_zero):
        return _np_reference(x, mask_np, np.asarray(sin), np.asarray(cos),
                             np.asarray(ln1_w), np.asarray(ln1_b),
                             np.asarray(w_qkv), np.asarray(w_proj),
                             np.asarray(ln2_w), np.asarray(ln2_b),
                             np.asarray(w_fc1), np.asarray(w_fc2))

    import jax
    try:
        jax.config.update("jax_compilation_cache_dir", "/tmp/jax_nc_cache")
        jax.config.update("jax_persistent_cache_min_compile_time_secs", 0.0)
        jax.config.update("jax_persistent_cache_min_entry_size_bytes", 0)
    except Exception:
        pass
    from concourse.bass_utils import run_bass_kernel_spmd
    nc = _build(debug=False)
    in_maps = _prep_inputs(x, np.asarray(sin, np.float32).reshape(S, HD),
                           np.asarray(cos, np.float32).reshape(S, HD),
                           np.asarray(ln1_w, np.float32),
                           np.asarray(w_qkv, np.float32),
                           np.asarray(w_proj, np.float32),
                           np.asarray(ln2_w, np.float32),
                           np.asarray(w_fc1, np.float32),
                           np.asarray(w_fc2, np.float32))
    res = run_bass_kernel_spmd(nc, in_maps, core_ids=list(range(NCORES)))
    return _assemble_output(res.results)


# revision 33
# speedup vs baseline: 1.5281x; 1.4974x over previous
"""Trainium2 Bass kernel for nn_Block_51178830299350 (dense transformer block).

Strategy (8 NeuronCores, single NEFF):
  - Head-tensor-parallel attention: 2 heads/core over all 4096 tokens.
  - LN1: per-core stats on own 512 tokens + tiny AllGather of rsqrt row.
    Mean-subtraction folded into pre-packed qkv weights (host side).
  - fp32r (TF32-like) matmuls everywhere: full PE rate, ~1e-4 rel err.
  - Flash-style transposed scores [k, q]; softmax denominator via a ones
    column appended to V; causal handled by loop bounds + 0/1 diagonal masks.
  - AllToAll (2MB/rank) to redistribute head-sharded attention output to
    token-sharded; proj + FFN run token-parallel with full weights streamed.
"""
import sys

sys.path.insert(0, "/opt/trn_rl_repo")

import numpy as np

B, S, D, H, HD = 2, 2048, 1024, 16, 64
INNER = 4 * D
NCORES = 8
TOK = B * S              # 4096 flat tokens
TOWN = TOK // NCORES     # 512 tokens per core
EPS = 1e-5
_BUILD_CACHE = {}


def round_fp32r(x: np.ndarray) -> np.ndarray:
    """Round fp32 -> fp32r (11-bit mantissa, RTNE), matching TRN2 PE input fmt."""
    v = np.ascontiguousarray(x, dtype=np.float32).view(np.uint32)
    low = v & np.uint32(0xFFF)
    half = np.uint32(0x800)
    rounded = (v & ~np.uint32(0xFFF)).copy()
    up = (low > half) | ((low == half) & (((v >> np.uint32(12)) & np.uint32(1)) != 0))
    rounded[up] += np.uint32(0x1000)
    return rounded.view(np.float32)


def _np_reference(x, mask, sin, cos, ln1_w, ln1_b, w_qkv, w_proj, ln2_w, ln2_b,
                  w_fc1, w_fc2):
    """Slow numpy fallback (only used if inputs violate kernel assumptions)."""
    from scipy.special import erf

    def ln(t, w, b):
        m = t.mean(-1, keepdims=True)
        v = ((t - m) ** 2).mean(-1, keepdims=True)
        return (t - m) / np.sqrt(v + EPS) * w + b

    def rope(t, sin, cos):
        half = t.shape[-1] // 2
        rot = np.concatenate([-t[..., half:], t[..., :half]], -1)
        return t * cos + rot * sin

    b, s, d = x.shape
    hx = ln(x, ln1_w, ln1_b)
    qkv = (hx @ w_qkv).reshape(b, s, 3, H, HD).transpose(2, 0, 3, 1, 4)
    q, k, v = qkv[0], qkv[1], qkv[2]
    q = rope(q, sin, cos)
    k = rope(k, sin, cos)
    att = np.einsum("bhqd,bhkd->bhqk", q, k) / np.sqrt(HD)
    att = np.where(mask, att, -np.inf)
    att = att - att.max(-1, keepdims=True)
    p = np.exp(att)
    p /= p.sum(-1, keepdims=True)
    o = np.einsum("bhqk,bhkd->bhqd", p, v)
    o = o.transpose(0, 2, 1, 3).reshape(b, s, d) @ w_proj
    x = x + o
    h2 = ln(x, ln2_w, ln2_b)
    h2 = h2 @ w_fc1
    h2 = 0.5 * h2 * (1.0 + erf(h2 / np.sqrt(2.0)))
    h2 = h2 @ w_fc2
    return (x + h2).astype(np.float32)


def _build(debug=False, sim=False):
    key = ("nc", debug, sim)
    if key in _BUILD_CACHE:
        return _BUILD_CACHE[key]
    import concourse.bacc as bacc
    import concourse.bass as bass_mod
    import concourse.tile as tile
    from concourse import mybir

    F32 = mybir.dt.float32
    F32R = mybir.dt.float32r
    AF = mybir.ActivationFunctionType

    nc = bacc.Bacc("TRN2", target_bir_lowering=False, debug=False,
                   enable_asserts=False, num_devices=NCORES)

    # ---------------- DRAM parameters (per core) ----------------
    # packed transposed x: [128, (t*8+dt)*512]; (p, t, dt, i) = xT[dt*128+p, t*512+i]
    xtp = nc.dram_tensor("xtp", [128, 8 * 8 * 512], mybir.dt.bfloat16,
                         kind="ExternalInput")
    # own-slice xT (pre-rounded) for stats + residual: [128, dt*512 + i]
    xres = nc.dram_tensor("xres", [128, 8 * 512], F32R, kind="ExternalInput")
    # qkv weights for this core's 2 heads: [128, dt*384 + {Q(128)|K(128)|V(128)}]
    wqkv = nc.dram_tensor("wqkv", [128, 8 * 384], mybir.dt.bfloat16,
                          kind="ExternalInput")
    # proj weights (full): [128, (jb*8+dt)*128 + j]
    wproj = nc.dram_tensor("wproj", [128, 8 * 8 * 128], mybir.dt.bfloat16, kind="ExternalInput")
    # fc1 (full, ln2_w folded): [128, (j*8+dt)*128 + jj]
    wfc1 = nc.dram_tensor("wfc1", [128, 32 * 8 * 128], mybir.dt.bfloat16, kind="ExternalInput")
    # fc2 (full): [128, (d*32+jt)*128 + dd]
    wfc2 = nc.dram_tensor("wfc2", [128, 8 * 32 * 128], mybir.dt.bfloat16, kind="ExternalInput")
    # rope tables [128, S]: head-dim table stacked twice (both local heads),
    # transposed, sign-folded sin (batch-independent)
    cosr = nc.dram_tensor("cosr", [128, S], F32, kind="ExternalInput")
    sinr = nc.dram_tensor("sinr", [128, S], F32, kind="ExternalInput")
    # 4 canonical 0/1 diagonal mask tiles [128k, 512q]
    maskt = nc.dram_tensor("maskt", [128, 4 * 512], F32R, kind="ExternalInput")
    # constants
    ones128 = nc.dram_tensor("ones128", [128, 128], F32R, kind="ExternalInput")  # 1/1024
    ident2 = nc.dram_tensor("ident2", [128, 64], F32, kind="ExternalInput")      # eye64 x2
    onescol = nc.dram_tensor("onescol", [128, 16], F32R, kind="ExternalInput")   # 1.0
    brow = nc.dram_tensor("brow", [1, 128], F32R, kind="ExternalInput")          # 1.0

    outT = nc.dram_tensor("outT", [128, 8 * 512], F32, kind="ExternalOutput")

    # collective bounce buffers
    ag_in = nc.dram_tensor("ag_in", [1, TOWN], F32)
    ag_out = nc.dram_tensor("ag_out", [NCORES, TOWN], F32, addr_space="Shared")
    a2a_in = [nc.dram_tensor(f"a2a_in{h}", [NCORES, 64, TOWN], mybir.dt.bfloat16)
              for h in range(2)]
    a2a_out = [nc.dram_tensor(f"a2a_out{h}", [NCORES, 64, TOWN], mybir.dt.bfloat16)
               for h in range(2)]

    dbg = {}
    if debug:
        for name, shape in [("dbg_q", [128, TOK]),
                            ("dbg_k", [128, TOK]), ("dbg_vtok", [128, 16 * 65]),
                            ("dbg_rs", [1, TOK]), ("dbg_pt", [128, 512]),
                            ("dbg_att", [128, 8 * 512]), ("dbg_x2", [128, 8 * 512]),
                            ("dbg_h1", [128, 4 * 512]), ("dbg_rs2", [1, TOWN]),
                            ("dbg_m2", [1, TOWN])]:
            dbg[name] = nc.dram_tensor(name, shape, F32, kind="ExternalOutput")

    RG = [list(range(NCORES))]

    def bc_ap(dram, nparts, ncols, offset=0):
        """partition-broadcast read AP over a DRAM row."""
        return bass_mod.AP(tensor=dram.ap().tensor, offset=offset,
                           ap=[[0, nparts], [1, ncols]])

    with tile.TileContext(nc) as tc:
        import contextlib
        with contextlib.ExitStack() as ctx:
            consts = ctx.enter_context(tc.tile_pool(name="consts", bufs=1))
            xrpool = ctx.enter_context(tc.tile_pool(name="xrpool", bufs=1))
            qkctx = contextlib.ExitStack()
            qkpool = qkctx.enter_context(tc.tile_pool(name="qk", bufs=1))
            vpool = qkctx.enter_context(tc.tile_pool(name="vtok", bufs=1))
            attsb = qkctx.enter_context(tc.tile_pool(name="attsb", bufs=2))
            attps = qkctx.enter_context(
                tc.tile_pool(name="attps", bufs=2, space="PSUM"))
            ph1ps_ctx = contextlib.ExitStack()
            ph1ps = ph1ps_ctx.enter_context(
                tc.tile_pool(name="ph1ps", bufs=2, space="PSUM"))

            ones_t = consts.tile([128, 128], F32R)
            nc.sync.dma_start(out=ones_t[:], in_=ones128[:, :])
            mask_t = consts.tile([128, 4, 512], F32R)
            nc.sync.dma_start(out=mask_t[:],
                              in_=maskt[:, :].rearrange("p (o q) -> p o q", o=4))
            eps_t = consts.tile([1, 1], F32)
            nc.vector.memset(eps_t[:], EPS)
            brow_t = consts.tile([1, 128], F32R)
            nc.sync.dma_start(out=brow_t[:], in_=brow[:, :])

            # persistent Q', K' (feature-major, 2 local heads x 64 dims)
            q_t = qkpool.tile([128, TOK], F32R)
            k_t = qkpool.tile([128, TOK], F32R)
            v_tok = [[vpool.tile([128, 16, 65], F32R, tag=f"vtok{b}{h}",
                                 name=f"vtok{b}{h}")
                      for h in range(2)] for b in range(B)]

            def attn_jt(hl, b, jt):
                o = hl * 64
                nkt = 4 * (jt + 1)
                po = attps.tile([65, 512], F32, tag="po", name="po", bufs=2)
                qs = q_t[o:o + 64, b * S + jt * 512: b * S + (jt + 1) * 512]
                for kt in range(nkt):
                    ps = attps.tile([128, 512], F32, tag="ps", name="ps", bufs=3)
                    ks = k_t[o:o + 64, b * S + kt * 128: b * S + (kt + 1) * 128]
                    nc.tensor.matmul(ps[:], ks, qs, start=True, stop=True)
                    pt = attsb.tile([128, 512], F32R, tag="pt", name="pt", bufs=4)
                    nc.scalar.activation(pt[:], ps[:], AF.Exp)
                    od = kt - 4 * jt
                    if od >= 0:
                        nc.vector.tensor_mul(pt[:], pt.bitcast(F32)[:],
                                             mask_t.bitcast(F32)[:, od, :])
                    if debug and b == 0 and hl == 0 and jt == 0 and kt == 0:
                        nc.sync.dma_start(out=dbg["dbg_pt"][:, :],
                                          in_=pt.bitcast(F32)[:])
                    nc.tensor.matmul(po[:], v_tok[b][hl][:, kt, :], pt[:],
                                     start=(kt == 0), stop=(kt == nkt - 1))
                rcp = attsb.tile([1, 512], F32R, tag="rcp", name="rcp")
                with nc.allow_low_precision(reason="recip bcast f32r"):
                    nc.vector.reciprocal(rcp[:], po[64:65, :])
                psb = attps.tile([64, 512], F32, tag="ps", name="psb", bufs=3)
                nc.tensor.matmul(psb[:], brow_t[0:1, 0:64], rcp[:],
                                 start=True, stop=True)
                rb = attsb.tile([64, 512], F32, tag="rb", name="rb")
                nc.vector.tensor_copy(rb[:], psb[:])
                ov = attsb.tile([64, 512], mybir.dt.bfloat16, tag="ov", name="ov")
                nc.vector.tensor_mul(ov[:], po[0:64, :], rb[:])
                nc.sync.dma_start(out=a2a_in[hl][b * 4 + jt, :, :], in_=ov[:])

            xown = xrpool.tile([128, 8, 512], F32R)
            # ================= phase 1a: LN1 stats + AllGather =================
            with tc.tile_pool(name="ph1a", bufs=1) as ph1a:
                for dt in range(8):
                    nc.sync.dma_start(out=xown[:, dt, :],
                                      in_=xres[:, dt * 512:(dt + 1) * 512])
                sq = ph1a.tile([128, 8, 512], F32R)
                nc.vector.tensor_mul(sq[:], xown.bitcast(F32)[:], xown.bitcast(F32)[:])
                mps = ph1ps.tile([128, 512], F32, tag="qkvps")
                sps = ph1ps.tile([128, 512], F32, tag="qkvps")
                for dt in range(8):
                    nc.tensor.matmul(mps[:], ones_t[:], xown[:, dt, :],
                                     start=(dt == 0), stop=(dt == 7))
                for dt in range(8):
                    nc.tensor.matmul(sps[:], ones_t[:], sq[:, dt, :],
                                     start=(dt == 0), stop=(dt == 7))
                mrow = ph1a.tile([1, 512], F32)
                nc.vector.tensor_copy(mrow[:], mps[0:1, :])
                msq = ph1a.tile([1, 512], F32)
                nc.vector.tensor_copy(msq[:], sps[0:1, :])
                var = ph1a.tile([1, 512], F32)
                nc.vector.tensor_mul(var[:], mrow[:], mrow[:])
                nc.vector.tensor_sub(var[:], msq[:], var[:])
                rsq = ph1a.tile([1, 512], F32)
                nc.scalar.activation(rsq[:], var[:], AF.Sqrt, bias=eps_t[:])
                rs_own = ph1a.tile([1, 512], F32)
                nc.vector.reciprocal(rs_own[:], rsq[:])
                nc.sync.dma_start(out=ag_in[:, :], in_=rs_own[:])
                if sim:
                    nc.sync.dma_start(out=ag_out[:, :],
                                      in_=bc_ap(ag_in, NCORES, TOWN))
                else:
                    nc.gpsimd.collective_compute(
                        "AllGather", mybir.AluOpType.bypass, replica_groups=RG,
                        ins=[ag_in.ap().opt()], outs=[ag_out.ap().opt()])
                if debug:
                    nc.sync.dma_start(
                        out=dbg["dbg_rs"][:, :],
                        in_=ag_out[:, :].rearrange("r t -> (r t)")
                        .rearrange("(o t) -> o t", o=1))

            # ====== phase 1b: qkv matmuls + RoPE + V fixup, per 512-token tile =
            with tc.tile_pool(name="ph1b", bufs=1) as ph1b, \
                 tc.tile_pool(name="ph1x", bufs=2) as ph1x:
                id_t = ph1b.tile([128, 64], F32)
                nc.sync.dma_start(out=id_t[:], in_=ident2[:, :])
                ocol_t = ph1b.tile([128, 16], F32R)
                nc.sync.dma_start(out=ocol_t[:], in_=onescol[:, :])
                cos_t = ph1b.tile([128, S], F32)
                nc.sync.dma_start(out=cos_t[:], in_=cosr[:, :])
                sin_t = ph1b.tile([128, S], F32)
                nc.sync.dma_start(out=sin_t[:], in_=sinr[:, :])
                rs_col = ph1b.tile([128, 32], F32)
                nc.sync.dma_start(
                    out=rs_col[:],
                    in_=bass_mod.AP(tensor=ag_out.ap().tensor, offset=0,
                                    ap=[[1, 128], [128, 32]]))
                wq_t = ph1b.tile([128, 8, 384], mybir.dt.bfloat16)
                nc.sync.dma_start(out=wq_t[:],
                                  in_=wqkv[:, :].rearrange("p (d c) -> p d c", d=8))
                for b in range(B):
                    for hl in range(2):
                        nc.sync.dma_start(
                            out=v_tok[b][hl][:, :, 64:65],
                            in_=ocol_t[:].rearrange("p (s o) -> p s o", o=1))

                for t in range(8):
                    b, soff = t // 4, (t % 4) * 512
                    tsl = slice(t * 512, (t + 1) * 512)
                    ssl = slice(soff, soff + 512)
                    xt = ph1x.tile([128, 8, 512], mybir.dt.bfloat16, tag="xt",
                                   bufs=4)
                    for dt in range(8):
                        nc.sync.dma_start(
                            out=xt[:, dt, :],
                            in_=xtp[:, t * 4096 + dt * 512: t * 4096 + (dt + 1) * 512])
                    v_raw = ph1x.tile([128, 512], F32, tag="vraw", bufs=3)
                    raws = [(q_t, None), (k_t, None), (None, v_raw)]
                    for jb in range(3):
                        pj = ph1ps.tile([128, 512], F32, tag="qkvps")
                        for dt in range(8):
                            nc.tensor.matmul(
                                pj[:], wq_t[:, dt, jb * 128:(jb + 1) * 128],
                                xt[:, dt, :], start=(dt == 0), stop=(dt == 7))
                        big, small = raws[jb]
                        if big is not None:
                            nc.vector.tensor_copy(big[:, tsl], pj[:])
                        else:
                            nc.vector.tensor_copy(small[:], pj[:])
                    # RoPE on this token tile (both heads in one op)
                    rs_c = ph1x.tile([128, 512], F32, tag="rsc", bufs=3)
                    nc.sync.dma_start(out=rs_c[:],
                                      in_=bc_ap(ag_out, 128, 512, offset=t * 512))
                    for tn in (q_t, k_t):
                        swp = ph1x.tile([128, 512], F32, tag="swp", bufs=3)
                        for hl in range(2):
                            o = hl * 64
                            nc.sync.dma_start(out=swp[o:o + 32, :],
                                              in_=tn.bitcast(F32)[o + 32:o + 64, tsl])
                            nc.sync.dma_start(out=swp[o + 32:o + 64, :],
                                              in_=tn.bitcast(F32)[o:o + 32, tsl])
                        ta = ph1x.tile([128, 512], F32, tag="ropea")
                        tb = ph1x.tile([128, 512], F32, tag="ropeb")
                        nc.vector.tensor_mul(ta[:], tn.bitcast(F32)[:, tsl],
                                             cos_t[:, ssl])
                        nc.vector.tensor_mul(tb[:], swp[:], sin_t[:, ssl])
                        nc.vector.tensor_add(ta[:], ta[:], tb[:])
                        nc.vector.tensor_mul(tn[:, tsl], ta[:], rs_c[:])
                    # V transposes for this tile (4 x 128 tokens, each head)
                    for hl in range(2):
                        for ktl in range(4):
                            kt = (t % 4) * 4 + ktl
                            tp = ph1ps.tile([128, 64], F32, tag="trps", bufs=1)
                            nc.tensor.transpose(
                                tp[:], v_raw[hl * 64:(hl + 1) * 64,
                                             ktl * 128:(ktl + 1) * 128],
                                id_t[hl * 64:(hl + 1) * 64, :])
                            nc.scalar.activation(
                                v_tok[b][hl][:, kt, 0:64], tp[:], AF.Copy,
                                scale=rs_col[:, b * 16 + kt: b * 16 + kt + 1])
                    attn_jt(0, b, t % 4)
                if debug:
                    nc.sync.dma_start(out=dbg["dbg_q"][:, :],
                                      in_=q_t.bitcast(F32)[:])
                    nc.sync.dma_start(out=dbg["dbg_k"][:, :],
                                      in_=k_t.bitcast(F32)[:])
                    nc.sync.dma_start(out=dbg["dbg_vtok"][:, :],
                                      in_=v_tok[0][0].bitcast(F32)[:]
                                      .rearrange("p a b -> p (a b)"))
            ph1ps_ctx.close()

            # ==== phase 2 remainder: second head attention + both AllToAlls ====
            if sim:
                nc.sync.dma_start(out=a2a_out[0].ap(), in_=a2a_in[0].ap())
            else:
                nc.gpsimd.collective_compute(
                    "AllToAll", mybir.AluOpType.bypass, replica_groups=RG,
                    ins=[a2a_in[0].ap().opt()], outs=[a2a_out[0].ap().opt()])
            for b in range(B):
                for jt in range(4):
                    attn_jt(1, b, jt)
            if sim:
                nc.sync.dma_start(out=a2a_out[1].ap(), in_=a2a_in[1].ap())
            else:
                nc.gpsimd.collective_compute(
                    "AllToAll", mybir.AluOpType.bypass, replica_groups=RG,
                    ins=[a2a_in[1].ap().opt()], outs=[a2a_out[1].ap().opt()])
            qkctx.close()  # free Q/K/V + attention SBUF/PSUM before the FFN phase

            # ============ phase 3: proj + residual + LN2 + FFN =================
            with tc.tile_pool(name="ph3", bufs=1) as ph3, \
                 tc.tile_pool(name="ph3w", bufs=2) as ph3w, \
                 tc.tile_pool(name="ph3ps", bufs=2, space="PSUM") as ph3ps:
                attn_t = ph3.tile([128, 8, 512], mybir.dt.bfloat16, tag="bigA")
                for hl in range(2):
                    nc.sync.dma_start(
                        out=attn_t[hl * 64:(hl + 1) * 64, :, :],
                        in_=a2a_out[hl].ap().rearrange("i f t -> f i t"))
                if debug:
                    atf = ph3w.tile([128, 8, 512], F32, tag="atf")
                    nc.vector.tensor_copy(atf[:], attn_t[:])
                    nc.sync.dma_start(out=dbg["dbg_att"][:, :],
                                      in_=atf[:].rearrange("p a b -> p (a b)"))
                x2 = ph3.tile([128, 8, 512], F32R)
                for jb in range(8):
                    wpc = ph3w.tile([128, 8, 128], mybir.dt.bfloat16, tag="wpc")
                    nc.sync.dma_start(
                        out=wpc[:], in_=wproj[:, jb * 1024:(jb + 1) * 1024]
                        .rearrange("p (d c) -> p d c", d=8))
                    pp = ph3ps.tile([128, 512], F32, tag="pp")
                    for dt in range(8):
                        nc.tensor.matmul(pp[:], wpc[:, dt, :], attn_t[:, dt, :],
                                         start=(dt == 0), stop=(dt == 7))
                    nc.vector.tensor_add(x2[:, jb, :], pp[:],
                                         xown.bitcast(F32)[:, jb, :])
                if debug:
                    nc.sync.dma_start(out=dbg["dbg_x2"][:, :],
                                      in_=x2.bitcast(F32)[:]
                                      .rearrange("p a b -> p (a b)"))

                # LN2 stats (local, own tokens)
                sq2 = ph3.tile([128, 8, 512], F32R, tag="bigB")
                nc.vector.tensor_mul(sq2[:], x2.bitcast(F32)[:], x2.bitcast(F32)[:])
                mps2 = ph3ps.tile([128, 512], F32, tag="mps2", bufs=1)
                sps2 = ph3ps.tile([128, 512], F32, tag="sps2", bufs=1)
                for dt in range(8):
                    nc.tensor.matmul(mps2[:], ones_t[:], x2[:, dt, :],
                                     start=(dt == 0), stop=(dt == 7))
                for dt in range(8):
                    nc.tensor.matmul(sps2[:], ones_t[:], sq2[:, dt, :],
                                     start=(dt == 0), stop=(dt == 7))
                m2 = ph3.tile([1, 512], F32)
                nc.vector.tensor_copy(m2[:], mps2[0:1, :])
                msq2 = ph3.tile([1, 512], F32)
                nc.vector.tensor_copy(msq2[:], sps2[0:1, :])
                var2 = ph3.tile([1, 512], F32)
                nc.vector.tensor_mul(var2[:], m2[:], m2[:])
                nc.vector.tensor_sub(var2[:], msq2[:], var2[:])
                rsq2 = ph3.tile([1, 512], F32)
                nc.scalar.activation(rsq2[:], var2[:], AF.Sqrt, bias=eps_t[:])
                rs2 = ph3.tile([1, 512], F32R)
                with nc.allow_low_precision(reason="rs2 bcast f32r"):
                    nc.vector.reciprocal(rs2[:], rsq2[:])
                if debug:
                    nc.sync.dma_start(out=dbg["dbg_rs2"][:, :],
                                      in_=rs2.bitcast(F32)[:])
                    nc.sync.dma_start(out=dbg["dbg_m2"][:, :], in_=m2[:])
                m2r = ph3.tile([1, 512], F32R)
                nc.vector.tensor_copy(m2r[:], m2[:])
                m2b = ph3ps.tile([128, 512], F32, tag="pp")
                nc.tensor.matmul(m2b[:], brow_t[:], m2r[:], start=True, stop=True)
                rs2b = ph3ps.tile([128, 512], F32, tag="pp")
                nc.tensor.matmul(rs2b[:], brow_t[:], rs2[:], start=True, stop=True)
                x2n = ph3.tile([128, 8, 512], mybir.dt.bfloat16)
                for dt in range(8):
                    tnrm = ph3w.tile([128, 512], F32, tag="tnrm")
                    nc.vector.tensor_sub(tnrm[:], x2.bitcast(F32)[:, dt, :], m2b[:])
                    nc.vector.tensor_mul(x2n[:, dt, :], tnrm[:], rs2b[:])

                # fc1 + gelu
                h1 = ph3.tile([128, 32, 512], mybir.dt.bfloat16)
                for jg in range(16):  # 16 chunks of 2 j-tiles
                    wc = ph3w.tile([128, 2048], mybir.dt.bfloat16, tag="wc")
                    nc.sync.dma_start(out=wc[:],
                                      in_=wfc1[:, jg * 2048:(jg + 1) * 2048])
                    for jj in range(2):
                        j = jg * 2 + jj
                        pf = ph3ps.tile([128, 512], F32, tag="pf")
                        for dt in range(8):
                            nc.tensor.matmul(
                                pf[:],
                                wc[:, (jj * 8 + dt) * 128:(jj * 8 + dt + 1) * 128],
                                x2n[:, dt, :], start=(dt == 0), stop=(dt == 7))
                        nc.scalar.activation(h1[:, j, :], pf[:], AF.Gelu)
                if debug:
                    h1f = ph3w.tile([128, 4, 512], F32, tag="h1f")
                    nc.vector.tensor_copy(h1f[:], h1[:, 0:4, :])
                    nc.sync.dma_start(out=dbg["dbg_h1"][:, :],
                                      in_=h1f[:].rearrange("p a b -> p (a b)"))

                # fc2 + residual + out
                for d in range(8):
                    pf2 = ph3ps.tile([128, 512], F32, tag="pf2")
                    for half in range(2):
                        wc2 = ph3w.tile([128, 2048], mybir.dt.bfloat16, tag="wc")
                        nc.sync.dma_start(
                            out=wc2[:],
                            in_=wfc2[:, (d * 32 + half * 16) * 128:
                                     (d * 32 + (half + 1) * 16) * 128])
                        for jj in range(16):
                            jt = half * 16 + jj
                            nc.tensor.matmul(pf2[:], wc2[:, jj * 128:(jj + 1) * 128],
                                             h1[:, jt, :], start=(jt == 0),
                                             stop=(jt == 31))
                    ot = ph3w.tile([128, 512], F32, tag="ot")
                    nc.vector.tensor_add(ot[:], pf2[:], x2.bitcast(F32)[:, d, :])
                    nc.sync.dma_start(out=outT[:, d * 512:(d + 1) * 512], in_=ot[:])

    nc.compile()
    _BUILD_CACHE[key] = nc
    return nc


def _prep_inputs(x, sin, cos, ln1_w, w_qkv, w_proj, ln2_w, w_fc1, w_fc2):
    """Host-side packing/folding. Returns in_maps (list of 8 dicts)."""
    import ml_dtypes
    xf = np.ascontiguousarray(x.reshape(TOK, D).T)          # [1024, 4096]
    xtp_r = round_fp32r(xf)
    xbf = xf.astype(ml_dtypes.bfloat16)
    xtp = np.empty((128, 8 * 8 * 512), ml_dtypes.bfloat16)
    for t in range(8):
        for dt in range(8):
            xtp[:, (t * 8 + dt) * 512:(t * 8 + dt + 1) * 512] = \
                xbf[dt * 128:(dt + 1) * 128, t * 512:(t + 1) * 512]

    # qkv weight fold: ln1_w scale, q-scale 1/8, mean-subtraction fold
    w1 = (ln1_w[:, None] * w_qkv).astype(np.float64)
    w1[:, :D] *= 1.0 / np.sqrt(HD)
    w1 = w1 - w1.mean(axis=0, keepdims=True)
    w1 = w1.astype(np.float32)

    wp = w_proj.astype(ml_dtypes.bfloat16)
    wproj_p = np.empty((128, 8 * 8 * 128), ml_dtypes.bfloat16)
    for jb in range(8):
        for dt in range(8):
            wproj_p[:, (jb * 8 + dt) * 128:(jb * 8 + dt + 1) * 128] = \
                wp[dt * 128:(dt + 1) * 128, jb * 128:(jb + 1) * 128]

    wf1 = (ln2_w[:, None] * w_fc1).astype(ml_dtypes.bfloat16)  # [1024, 4096]
    wfc1_p = np.empty((128, 32 * 8 * 128), ml_dtypes.bfloat16)
    for j in range(32):
        for dt in range(8):
            wfc1_p[:, (j * 8 + dt) * 128:(j * 8 + dt + 1) * 128] = \
                wf1[dt * 128:(dt + 1) * 128, j * 128:(j + 1) * 128]
    wf2 = w_fc2.astype(ml_dtypes.bfloat16)                   # [4096, 1024]
    wfc2_p = np.empty((128, 8 * 32 * 128), ml_dtypes.bfloat16)
    for d in range(8):
        for jt in range(32):
            wfc2_p[:, (d * 32 + jt) * 128:(d * 32 + jt + 1) * 128] = \
                wf2[jt * 128:(jt + 1) * 128, d * 128:(d + 1) * 128]

    cos1 = np.ascontiguousarray(cos.reshape(S, HD).T, dtype=np.float32)  # [64, S]
    sin1 = np.ascontiguousarray(sin.reshape(S, HD).T, dtype=np.float32)
    sin1[:HD // 2, :] *= -1.0
    cosf = np.concatenate([cos1, cos1], axis=0)  # [128, S]: both local heads
    sinT = np.concatenate([sin1, sin1], axis=0)

    maskt = np.zeros((128, 4, 512), np.float32)
    rk = np.arange(128)[:, None]
    rq = np.arange(512)[None, :]
    for o in range(4):
        maskt[:, o, :] = (128 * o + rk <= rq).astype(np.float32)
    maskt = maskt.reshape(128, 4 * 512)

    ones128 = np.full((128, 128), 1.0 / D, np.float32)  # 2^-10, fp32r exact
    brow = np.ones((1, 128), np.float32)
    ident2 = np.concatenate([np.eye(64, dtype=np.float32)] * 2, axis=0)
    onescol = np.ones((128, 16), np.float32)

    in_maps = []
    for c in range(NCORES):
        heads = [2 * c, 2 * c + 1]
        cols = []
        for blk in range(3):  # Q, K, V
            for h in heads:
                cols.extend(range(blk * D + h * HD, blk * D + (h + 1) * HD))
        wsel = w1[:, cols].astype(ml_dtypes.bfloat16)  # [1024, 384]
        wqkv_p = np.empty((128, 8 * 384), ml_dtypes.bfloat16)
        for dt in range(8):
            wqkv_p[:, dt * 384:(dt + 1) * 384] = wsel[dt * 128:(dt + 1) * 128, :]
        xres = np.empty((128, 8 * 512), np.float32)
        xslice = xtp_r[:, c * TOWN:(c + 1) * TOWN]  # [1024, 512] pre-rounded
        for dt in range(8):
            xres[:, dt * 512:(dt + 1) * 512] = xslice[dt * 128:(dt + 1) * 128, :]
        in_maps.append({
            "xtp": xtp, "xres": xres, "wqkv": wqkv_p, "wproj": wproj_p,
            "wfc1": wfc1_p, "wfc2": wfc2_p, "cosr": cosf, "sinr": sinT,
            "maskt": maskt, "ones128": ones128, "ident2": ident2,
            "onescol": onescol, "brow": brow,
        })
    return in_maps


def _assemble_output(results):
    full = np.empty((TOK, D), np.float32)
    for c in range(NCORES):
        blk = results[c]["outT"].reshape(128, 8, 512)
        for d in range(8):
            full[c * TOWN:(c + 1) * TOWN, d * 128:(d + 1) * 128] = blk[:, d, :].T
    return full.reshape(B, S, D)


def kernel(x, mask, sin, cos, ln1_w, ln1_b, w_qkv, w_proj, ln2_w, ln2_b,
           w_fc1, w_fc2):
    x = np.asarray(x, np.float32)
    mask_np = np.asarray(mask)
    causal = np.array_equal(
        mask_np.reshape(S, S), np.tril(np.ones((S, S), dtype=bool)))
    biases_zero = (np.abs(np.asarray(ln1_b)).max() == 0.0 and
                   np.abs(np.asarray(ln2_b)).max() == 0.0)
    if not (causal and biases_zero):
        return _np_reference(x, mask_np, np.asarray(sin), np.asarray(cos),
                             np.asarray(ln1_w), np.asarray(ln1_b),
                             np.asarray(w_qkv), np.asarray(w_proj),
                             np.asarray(ln2_w), np.asarray(ln2_b),
                             np.asarray(w_fc1), np.asarray(w_fc2))

    import jax
    try:
        jax.config.update("jax_compilation_cache_dir", "/tmp/jax_nc_cache")
        jax.config.update("jax_persistent_cache_min_compile_time_secs", 0.0)
        jax.config.update("jax_persistent_cache_min_entry_size_bytes", 0)
    except Exception:
        pass
    from concourse.bass_utils import run_bass_kernel_spmd
    nc = _build(debug=False)
    in_maps = _prep_inputs(x, np.asarray(sin, np.float32).reshape(S, HD),
                           np.asarray(cos, np.float32).reshape(S, HD),
                           np.asarray(ln1_w, np.float32),
                           np.asarray(w_qkv, np.float32),
                           np.asarray(w_proj, np.float32),
                           np.asarray(ln2_w, np.float32),
                           np.asarray(w_fc1, np.float32),
                           np.asarray(w_fc2, np.float32))
    res = run_bass_kernel_spmd(nc, in_maps, core_ids=list(range(NCORES)))
    return _assemble_output(res.results)
